# revision 1
# baseline (speedup 1.0000x reference)
"""Trainium2 Bass kernel for nn_CrossAttention_45724221833727.

Data-parallel over batch: 8 samples -> 8 NeuronCores, one [S=2048, D=512]
cross-attention problem per core. Weights/pos replicated.

Per-core pipeline (fp32 throughout):
  A) load evo, PE-transpose to evoT [d, s] (zero-padded for the conv)
  B) depthwise conv1d along s in [d, s] layout (ACT muls + DVE adds)
  C) QT[e, i] = WqT-matmuls(evoT) + pos (PE-transpose folded into the same
     PSUM accumulation) + bq (K=1 matmul fold)
  D) KVT[o, s] and KV[s, o] both as matmuls of kvdw (pw_b folded as K=1
     matmul); ke = KV + evo (residual, in-place into evo tiles)
  E) attention: scoresT[j, i] -> exp (fused scale, no max-subtract; inputs
     are sub-unit-variance so |scores|*scale < ~3) -> PV + row-sum l
     accumulated in PSUM over j-blocks -> out = out_unnorm/l + ke
"""

import math

import numpy as np

import concourse.bass as bass
import concourse.mybir as mybir
import concourse.tile as tile
from concourse.bass_utils import run_bass_kernel_spmd
from concourse.masks import make_identity

F32 = mybir.dt.float32
F32R = mybir.dt.float32r
P = 128
S = 2048
D = 512
KS = 6
N_CORES = 8
SB = S // P      # 16 s-blocks
DC = D // P      # 4 d-chunks
IT = S // 512    # 4 i-tiles of 512
ACT_EXP = mybir.ActivationFunctionType.Exp
ACT_COPY = mybir.ActivationFunctionType.Copy

_COMPILED = {}


def _install_tail_drain_patch():
    """This container's walrus build only accepts ONE sync wait per
    instruction; TileContext's tail drain carries one wait per live
    engine/DMA-queue. Split them across single-wait NOPs."""
    if getattr(tile.TileContext, "_tail_patch_installed", False):
        return

    def _patched_drain_and_barrier(self, tick_clock, wait_clock):
        from concourse.tile import ScopedClock

        drain_inst = self.nc.sync.drain()
        wait_clock.add_sem_waits(
            drain_inst.ins, ScopedClock({None: tick_clock.global_clock})
        )
        si = drain_inst.ins.sync_info
        waits = list(si.on_wait) if si and si.on_wait else []
        if len(waits) > 1:
            drain_inst.ins.sync_info = mybir.SyncInfo(
                on_wait=[], on_update=list(si.on_update or [])
            )
            for i, w in enumerate(waits):
                nop = self.nc.sync.nop(nofuse=True, hint=f"tail_wait_{i}")
                nop.ins.sync_info = mybir.SyncInfo(on_wait=[w], on_update=[])

        self.nc.all_engine_barrier()
        assert self.sems is not None
        popped = self.nc._tile_sem_poison_stack.pop()
        assert popped is self._sem_poison
        self.nc.clear_and_free_semaphores(list(self.sems.allocated().values()))
        self.nc.all_engine_barrier()

    tile.TileContext._drain_and_barrier = _patched_drain_and_barrier
    tile.TileContext._tail_patch_installed = True


def _split_multi_waits(nc):
    """Walrus in this container accepts at most ONE sync wait per
    instruction. Hoist extra waits onto single-wait NOPs inserted just
    before the instruction in the same engine's stream (equivalent
    semantics: the engine stalls at the NOP instead)."""
    ctr = [0]
    for fn in nc.m.functions:
        for blk in fn.blocks:
            insts = list(blk.instructions)
            out = []
            changed = False
            for inst in insts:
                si = inst.sync_info
                if si is not None and si.on_wait and len(si.on_wait) > 1:
                    waits = list(si.on_wait)
                    for w in waits[:-1]:
                        nop = mybir.InstNoOp(
                            name=f"splitw-{ctr[0]}", ins=[], outs=[]
                        )
                        ctr[0] += 1
                        nop.engine = inst.engine
                        nop.sync_info = mybir.SyncInfo(on_wait=[w], on_update=[])
                        out.append(nop)
                    inst.sync_info = mybir.SyncInfo(
                        on_wait=[waits[-1]], on_update=list(si.on_update or [])
                    )
                    changed = True
                out.append(inst)
            if changed:
                blk.instructions = out
    return nc


def _build():
    _install_tail_drain_patch()
    nc = bass.Bass()
    evo_d = nc.dram_tensor("evo", [S, D], F32, kind="ExternalInput")
    wqt_d = nc.dram_tensor("wqt", [D, D], F32R, kind="ExternalInput")   # Wq.T [d, e]
    bq_d = nc.dram_tensor("bq", [D], F32R, kind="ExternalInput")
    pos_d = nc.dram_tensor("pos", [S, D], F32, kind="ExternalInput")
    dww_d = nc.dram_tensor("dww", [D, KS], F32, kind="ExternalInput")  # depthwise taps
    pwt_d = nc.dram_tensor("pwt", [D, D], F32R, kind="ExternalInput")   # pw_w.T [d, o]
    pwb_d = nc.dram_tensor("pwb", [D], F32R, kind="ExternalInput")      # pw_b + pw_w@dw_b
    out_d = nc.dram_tensor("out", [S, D], F32, kind="ExternalOutput")

    scale = 1.0 / math.sqrt(float(D))
    PAD = 2056  # 2 left pad + 2048 + 3 right pad, rounded up

    with tile.TileContext(nc) as tc:
        # Long-lived pools on the LEFT side; phase-transient pools on the
        # RIGHT side, released LIFO so the stack allocator reclaims them.
        cpool = tc.alloc_tile_pool(name="consts", bufs=1, side="left")
        epool = tc.alloc_tile_pool(name="evo", bufs=6, side="left")

        # ---- constants ----
        ident = cpool.tile([P, P], F32, tag="ident")
        make_identity(nc, ident)
        ones_col = cpool.tile([P, 2], F32R, tag="ones_col")
        nc.vector.memset(ones_col[:].bitcast(F32), 1.0)
        ones_row = cpool.tile([1, 512], F32R, tag="ones_row")
        nc.vector.memset(ones_row[:].bitcast(F32), 1.0)
        bq_row = cpool.tile([1, D], F32R, tag="bq_row")
        nc.sync.dma_start(out=bq_row[:], in_=bq_d[None, :])
        pwb_row = cpool.tile([1, D], F32R, tag="pwb_row")
        nc.sync.dma_start(out=pwb_row[:], in_=pwb_d[None, :])
        bqT = cpool.tile([P, DC], F32, tag="bqT")
        nc.sync.dma_start(out=bqT[:], in_=bq_d.rearrange("(c p) -> p c", p=P).bitcast(F32))
        pwbT = cpool.tile([P, DC], F32, tag="pwbT")
        nc.sync.dma_start(out=pwbT[:], in_=pwb_d.rearrange("(c p) -> p c", p=P).bitcast(F32))
        pwb_bc = cpool.tile([P, D], F32, tag="pwb_bc")
        dww_sb = cpool.tile([P, DC * KS], F32, tag="dww")
        for dc in range(DC):
            nc.sync.dma_start(
                out=dww_sb[:, dc * KS:(dc + 1) * KS],
                in_=dww_d[dc * P:(dc + 1) * P, :],
            )
        wqt_sb = [cpool.tile([P, D], F32R, tag=f"wqt{dc}", name=f"wqt{dc}") for dc in range(DC)]
        pwt_sb = [cpool.tile([P, D], F32R, tag=f"pwt{dc}", name=f"pwt{dc}") for dc in range(DC)]
        for dc in range(DC):
            nc.sync.dma_start(out=wqt_sb[dc][:], in_=wqt_d[dc * P:(dc + 1) * P, :])
            nc.sync.dma_start(out=pwt_sb[dc][:], in_=pwt_d[dc * P:(dc + 1) * P, :])


        psb = tc.alloc_tile_pool(name="tpsB0", bufs=1, space="PSUM")
        ps_bc = psb.tile([P, D], F32, tag="psbc", name="psbc")
        nc.tensor.matmul(ps_bc[:], ones_row[0:1, :P], pwb_row[0:1, :],
                         start=True, stop=True)
        nc.scalar.copy(pwb_bc[:], ps_bc[:])
        psb.release()

        # ---- phases A-D, quarter-pipelined ----
        # For s-quarter q (512 wide): transpose the evo blocks it needs,
        # run the depthwise-conv tree for that quarter (taps split across
        # ACT and GpSimd, adds on DVE), emit the QT tile (it=q) and the
        # KVT/KV matmuls (st=q) as soon as their inputs exist.
        dwpool = tc.alloc_tile_pool(name="kvdw", bufs=8, side="right")
        etpool = tc.alloc_tile_pool(name="evoT", bufs=1, side="right")
        evoT = [etpool.tile([P, PAD], F32R, tag=f"evoT{dc}", name=f"evoT{dc}") for dc in range(DC)]

        qpool = tc.alloc_tile_pool(name="qt", bufs=1, side="left")
        qt = [qpool.tile([P, S], F32R, tag=f"qt{ec}", name=f"qt{ec}") for ec in range(DC)]
        kvtpool = tc.alloc_tile_pool(name="kvt", bufs=1, side="left")
        kvpool = tc.alloc_tile_pool(name="kv", bufs=1, side="left")
        kvt = [kvtpool.tile([P, S], F32R, tag=f"kvt{ob}", name=f"kvt{ob}") for ob in range(DC)]
        kv = [kvpool.tile([P, D], F32R, tag=f"kv{sb}", name=f"kv{sb}") for sb in range(SB)]

        psa = tc.alloc_tile_pool(name="tpsA", bufs=2, space="PSUM")
        psc = tc.alloc_tile_pool(name="tpsC", bufs=2, space="PSUM")
        psd = tc.alloc_tile_pool(name="tpsD", bufs=2, space="PSUM")
        dwtmp = tc.alloc_tile_pool(name="dwtmp", bufs=8, side="right")
        pospool = tc.alloc_tile_pool(name="pos", bufs=4, side="right")

        for dc in range(DC):
            nc.vector.memset(evoT[dc][:, 0:2].bitcast(F32), 0.0)
            nc.vector.memset(evoT[dc][:, 2 + S:PAD].bitcast(F32), 0.0)

        next_sb = 0
        for q in range(4):
            last_needed = min(4 * q + 4, SB - 1)
            while next_sb <= last_needed:
                sb = next_sb
                ev = epool.tile([P, D], F32, tag="evo", name="evo")
                nc.sync.dma_start(out=ev[:], in_=evo_d[sb * P:(sb + 1) * P, :])
                for dc in range(DC):
                    ps = psa.tile([P, P], F32, tag="tp", name="tp")
                    nc.tensor.transpose(
                        ps[:], ev[:, dc * P:(dc + 1) * P], ident[:]
                    )
                    nc.vector.tensor_copy(
                        evoT[dc][:, 2 + sb * P:2 + (sb + 1) * P], ps[:]
                    )
                next_sb += 1

            # depthwise conv for quarter q of each chunk: tree of 6 taps
            kvdw_q = [None] * DC
            for dc in range(DC):
                prods = []
                for k in range(KS):
                    tk = dwtmp.tile([P, 512], F32, tag="dwt", name="dwt")
                    eng = nc.scalar if (k % 2 == 0) else None
                    in_ap = evoT[dc][:, q * 512 + k:q * 512 + k + 512].bitcast(F32)
                    sc = dww_sb[:, dc * KS + k:dc * KS + k + 1]
                    if eng is nc.scalar:
                        nc.scalar.activation(tk[:], in_ap, ACT_COPY, scale=sc)
                    else:
                        nc.gpsimd.tensor_scalar(
                            tk[:], in_ap, sc, None, mybir.AluOpType.mult
                        )
                    prods.append(tk)
                a01 = dwtmp.tile([P, 512], F32, tag="dwt", name="dwa")
                nc.vector.tensor_add(a01[:], prods[0][:], prods[1][:])
                a23 = dwtmp.tile([P, 512], F32, tag="dwt", name="dwb")
                nc.vector.tensor_add(a23[:], prods[2][:], prods[3][:])
                a45 = dwtmp.tile([P, 512], F32, tag="dwt", name="dwc")
                nc.vector.tensor_add(a45[:], prods[4][:], prods[5][:])
                a03 = dwtmp.tile([P, 512], F32, tag="dwt", name="dwd")
                nc.vector.tensor_add(a03[:], a01[:], a23[:])
                kq = dwpool.tile([P, 512], F32R, tag="kvdwq", name="kvdwq")
                nc.vector.tensor_add(kq[:], a03[:], a45[:])
                kvdw_q[dc] = kq

            # phase C tile it=q (needs evoT columns of this quarter only)
            it = q
            pos_tiles = []
            for sub in range(4):
                pt_ = pospool.tile([P, D], F32, tag="pos", name="pos")
                sb = it * 4 + sub
                nc.sync.dma_start(out=pt_[:], in_=pos_d[sb * P:(sb + 1) * P, :])
                pos_tiles.append(pt_)
            for ec in range(DC):
                ps = psc.tile([P, 512], F32, tag="qps", name="qps")
                for dc in range(DC):
                    nc.tensor.matmul(
                        ps[:], wqt_sb[dc][:, ec * P:(ec + 1) * P],
                        evoT[dc][:, 2 + it * 512:2 + (it + 1) * 512],
                        start=(dc == 0), stop=False,
                    )
                for sub in range(4):
                    nc.tensor.matmul(
                        ps[:, sub * P:(sub + 1) * P],
                        pos_tiles[sub][:, ec * P:(ec + 1) * P],
                        ident[:], is_transpose=True,
                        start=False, stop=(sub == 3),
                    )
                nc.scalar.activation(
                    qt[ec][:, it * 512:(it + 1) * 512], ps[:],
                    mybir.ActivationFunctionType.Identity,
                    bias=bqT[:, ec:ec + 1],
                )

            # phase D for st=q
            st = q
            for ob in range(DC):
                ps = psd.tile([P, 512], F32, tag="dps", name="kvtps")
                for dc in range(DC):
                    nc.tensor.matmul(
                        ps[:], pwt_sb[dc][:, ob * P:(ob + 1) * P],
                        kvdw_q[dc][:],
                        start=(dc == 0), stop=(dc == DC - 1),
                    )
                nc.vector.tensor_scalar(
                    kvt[ob][:, st * 512:(st + 1) * 512], ps[:],
                    pwbT[:, ob:ob + 1], None, mybir.AluOpType.add,
                )
            for sb in range(4 * q, 4 * q + 4):
                ps = psd.tile([P, 512], F32, tag="dps", name="kvps")
                for dc in range(DC):
                    nc.tensor.matmul(
                        ps[:], kvdw_q[dc][:, (sb - 4 * q) * P:(sb - 4 * q + 1) * P],
                        pwt_sb[dc][:], start=(dc == 0), stop=(dc == DC - 1),
                    )
                nc.vector.tensor_add(kv[sb][:], ps[:], pwb_bc[:])

        pospool.release()
        dwtmp.release()
        etpool.release()
        dwpool.release()
        psd.release()
        psc.release()
        psa.release()

        # ---- phase E: attention ----
        pss = tc.alloc_tile_pool(name="spsE", bufs=2, space="PSUM")
        pso = tc.alloc_tile_pool(name="opsE", bufs=4, space="PSUM")
        psl = tc.alloc_tile_pool(name="lpsE", bufs=2, space="PSUM")
        ptpool = tc.alloc_tile_pool(name="ptE", bufs=3, side="right")
        epipool = tc.alloc_tile_pool(name="epi", bufs=4, side="right")
        for ig in range(IT):
            out_ps = [pso.tile([P, 512], F32, tag="ops", name="ops") for _ in range(4)]
            l_ps = psl.tile([P, 8], F32, tag="lps", name="lps")
            for jb in range(SB):
                s_ps = pss.tile([P, 512], F32, tag="sps", name="sps")
                for ec in range(DC):
                    nc.tensor.matmul(
                        s_ps[:], kvt[ec][:, jb * P:(jb + 1) * P],
                        qt[ec][:, ig * 512:(ig + 1) * 512],
                        start=(ec == 0), stop=(ec == DC - 1),
                    )
                p_t = ptpool.tile([P, 512], F32R, tag="pt", name="pt")
                nc.scalar.activation(p_t[:], s_ps[:], ACT_EXP, scale=scale)
                for ib in range(4):
                    nc.tensor.matmul(
                        out_ps[ib][:], p_t[:, ib * P:(ib + 1) * P],
                        kv[jb][:], start=(jb == 0), stop=(jb == SB - 1),
                    )
                    nc.tensor.matmul(
                        l_ps[:, 2 * ib:2 * ib + 2],
                        p_t[:, ib * P:(ib + 1) * P],
                        ones_col[:],
                        start=(jb == 0 and ib == 0),
                        stop=(jb == SB - 1 and ib == 3),
                    )
            for ib in range(4):
                sb = ig * 4 + ib
                rl = epipool.tile([P, 1], F32, tag="rl", name="rl")
                nc.vector.reciprocal(rl[:], l_ps[:, 2 * ib:2 * ib + 1])
                o_sb = epipool.tile([P, 512], F32, tag="osb", name="osb")
                nc.scalar.activation(
                    o_sb[:], out_ps[ib][:], ACT_COPY, scale=rl[:]
                )
                er = epipool.tile([P, 512], F32, tag="er", name="er")
                nc.sync.dma_start(out=er[:], in_=evo_d[sb * P:(sb + 1) * P, :])
                nc.vector.tensor_add(o_sb[:], o_sb[:], kv[sb][:].bitcast(F32))
                nc.vector.tensor_add(o_sb[:], o_sb[:], er[:])
                nc.sync.dma_start(out=out_d[sb * P:(sb + 1) * P, :], in_=o_sb[:])
        epipool.release()
        ptpool.release()
        psl.release()
        pso.release()
        pss.release()
        kvpool.release()
        kvtpool.release()
        qpool.release()
        epool.release()
        cpool.release()

    _split_multi_waits(nc)
    return nc


def kernel(evo_local, Wq, bq, dw_w, dw_b, pw_w, pw_b, pos):
    evo_local = np.asarray(evo_local, dtype=np.float32)
    Wq = np.asarray(Wq, dtype=np.float32)
    bq = np.asarray(bq, dtype=np.float32)
    dw_w = np.asarray(dw_w, dtype=np.float32)
    dw_b = np.asarray(dw_b, dtype=np.float32)
    pw_w = np.asarray(pw_w, dtype=np.float32)
    pw_b = np.asarray(pw_b, dtype=np.float32)
    pos = np.asarray(pos, dtype=np.float32)

    if "nc" not in _COMPILED:
        _COMPILED["nc"] = _build()
    nc = _COMPILED["nc"]

    wqt = np.ascontiguousarray(Wq.T)                      # [d, e]
    pwt = np.ascontiguousarray(pw_w.T)                    # [d, o]
    pwb_eff = (pw_b + pw_w @ dw_b).astype(np.float32)     # fold depthwise bias
    dww = np.ascontiguousarray(dw_w[:, 0, :])             # [D, K]
    pos0 = np.ascontiguousarray(pos[0])                   # [S, D]

    in_maps = []
    for c in range(N_CORES):
        in_maps.append({
            "evo": np.ascontiguousarray(evo_local[c]),
            "wqt": wqt,
            "bq": bq,
            "pos": pos0,
            "dww": dww,
            "pwt": pwt,
            "pwb": pwb_eff,
        })
    res = run_bass_kernel_spmd(nc, in_maps, core_ids=list(range(N_CORES)))
    out = np.stack([res.results[c]["out"] for c in range(N_CORES)], axis=0)
    return out.astype(np.float32)



# revision 6
# speedup vs baseline: 2.0291x; 2.0291x over previous
"""Trainium2 Bass kernel for nn_CrossAttention_45724221833727.

Data-parallel over batch: 8 samples -> 8 NeuronCores, one [S=2048, D=512]
cross-attention problem per core. Weights/pos replicated.

v2: everything dtype-bf16 on PE inputs; depthwise conv runs ON THE PE as
6 diagonal-matrix matmuls accumulating in PSUM (kills the GpSimd/DVE
elementwise conv that dominated v1); pos+bq folded host-side into a
transposed table added via an identity matmul; epilogue residuals folded
into the attention PSUM via l-scaled identity matmuls.

Per-core pipeline:
  A) load evo f32, ACT-cast to bf16, PE-transpose to evoT [d, s+pad]
  B) kvdw[d,s] = depthwise conv: sum_k diag(w_k) @ evoT(shift k) in PSUM
  C) QT[e, i] = Wq^T-matmuls(evoT) + ident-matmul(posT + bq)
  D) KVT[o, s] = pw^T-matmuls(kvdw) (+pwb via ACT bias);
     kv[s, o] = PE-transpose(KVT)
  E) attention: scoresT[j, i] -> exp (fused scale, no max-subtract;
     inputs are sub-unit-variance so |scores|*scale < ~3) -> PV + row-sum
     l in PSUM over j -> out_ps += l*kv + l*evo (ident matmuls)
     -> out = out_ps / l
"""

import math

import numpy as np
import ml_dtypes

import concourse.bass as bass
import concourse.mybir as mybir
import concourse.tile as tile
from concourse.bass_utils import run_bass_kernel_spmd
from concourse.masks import make_identity

F32 = mybir.dt.float32
F32R = mybir.dt.float32r
BF16 = mybir.dt.bfloat16
P = 128
S = 2048
D = 512
KS = 6
N_CORES = 8
SB = S // P      # 16 s-blocks
DC = D // P      # 4 d-chunks
IT = S // 512    # 4 i-tiles of 512
ACT_EXP = mybir.ActivationFunctionType.Exp
ACT_COPY = mybir.ActivationFunctionType.Copy
ACT_ID = mybir.ActivationFunctionType.Identity

_COMPILED = {}


def _install_tail_drain_patch():
    """This container's walrus build only accepts ONE sync wait per
    instruction; TileContext's tail drain carries one wait per live
    engine/DMA-queue. Split them across single-wait NOPs."""
    if getattr(tile.TileContext, "_tail_patch_installed", False):
        return

    def _patched_drain_and_barrier(self, tick_clock, wait_clock):
        from concourse.tile import ScopedClock

        drain_inst = self.nc.sync.drain()
        wait_clock.add_sem_waits(
            drain_inst.ins, ScopedClock({None: tick_clock.global_clock})
        )
        si = drain_inst.ins.sync_info
        waits = list(si.on_wait) if si and si.on_wait else []
        if len(waits) > 1:
            drain_inst.ins.sync_info = mybir.SyncInfo(
                on_wait=[], on_update=list(si.on_update or [])
            )
            for i, w in enumerate(waits):
                nop = self.nc.sync.nop(nofuse=True, hint=f"tail_wait_{i}")
                nop.ins.sync_info = mybir.SyncInfo(on_wait=[w], on_update=[])

        self.nc.all_engine_barrier()
        assert self.sems is not None
        popped = self.nc._tile_sem_poison_stack.pop()
        assert popped is self._sem_poison
        self.nc.clear_and_free_semaphores(list(self.sems.allocated().values()))
        self.nc.all_engine_barrier()

    tile.TileContext._drain_and_barrier = _patched_drain_and_barrier
    tile.TileContext._tail_patch_installed = True


def _split_multi_waits(nc):
    """Walrus in this container accepts at most ONE sync wait per
    instruction. Hoist extra waits onto single-wait NOPs inserted just
    before the instruction in the same engine's stream (equivalent
    semantics: the engine stalls at the NOP instead)."""
    ctr = [0]
    for fn in nc.m.functions:
        for blk in fn.blocks:
            insts = list(blk.instructions)
            out = []
            changed = False
            for inst in insts:
                si = inst.sync_info
                if si is not None and si.on_wait and len(si.on_wait) > 1:
                    waits = list(si.on_wait)
                    for w in waits[:-1]:
                        nop = mybir.InstNoOp(
                            name=f"splitw-{ctr[0]}", ins=[], outs=[]
                        )
                        ctr[0] += 1
                        nop.engine = inst.engine
                        nop.sync_info = mybir.SyncInfo(on_wait=[w], on_update=[])
                        out.append(nop)
                    inst.sync_info = mybir.SyncInfo(
                        on_wait=[waits[-1]], on_update=list(si.on_update or [])
                    )
                    changed = True
                out.append(inst)
            if changed:
                blk.instructions = out
    return nc


def _build():
    _install_tail_drain_patch()
    nc = bass.Bass()
    evo_d = nc.dram_tensor("evo", [S, D], F32, kind="ExternalInput")
    wqt_d = nc.dram_tensor("wqt", [D, D], BF16, kind="ExternalInput")   # Wq.T [d, e]
    post_d = nc.dram_tensor("post", [D, S], BF16, kind="ExternalInput")  # pos.T + bq [e, s]
    diagw_d = nc.dram_tensor("diagw", [DC * KS * P, P], BF16, kind="ExternalInput")
    pwt_d = nc.dram_tensor("pwt", [D, D], BF16, kind="ExternalInput")   # pw_w.T [d, o]
    pwb_d = nc.dram_tensor("pwb", [D], F32, kind="ExternalInput")       # pw_b + pw_w@dw_b
    idbf_d = nc.dram_tensor("idbf", [P, P], BF16, kind="ExternalInput")  # identity
    ones_d = nc.dram_tensor("ones", [P, 2], BF16, kind="ExternalInput")
    out_d = nc.dram_tensor("out", [S, D], F32, kind="ExternalOutput")

    scale = 1.0 / math.sqrt(float(D))
    PAD = 2056  # 2 left pad + 2048 + 3 right pad, rounded up

    with tile.TileContext(nc) as tc:
        cpool = tc.alloc_tile_pool(name="consts", bufs=1, side="left")
        epool = tc.alloc_tile_pool(name="evo", bufs=6, side="left")

        # ---- constants ----
        ident_bf = cpool.tile([P, P], BF16, tag="identbf")
        nc.sync.dma_start(out=ident_bf[:], in_=idbf_d[:, :])
        ones_col = cpool.tile([P, 2], BF16, tag="ones_col")
        nc.sync.dma_start(out=ones_col[:], in_=ones_d[:, :])
        pwbT = cpool.tile([P, DC], F32, tag="pwbT")
        nc.sync.dma_start(out=pwbT[:], in_=pwb_d.rearrange("(c p) -> p c", p=P))
        diagw_sb = [
            cpool.tile([P, P], BF16, tag=f"dg{t}", name=f"dg{t}")
            for t in range(DC * KS)
        ]
        for t in range(DC * KS):
            nc.sync.dma_start(
                out=diagw_sb[t][:], in_=diagw_d[t * P:(t + 1) * P, :]
            )
        wqt_sb = [cpool.tile([P, D], BF16, tag=f"wqt{dc}", name=f"wqt{dc}") for dc in range(DC)]
        pwt_sb = [cpool.tile([P, D], BF16, tag=f"pwt{dc}", name=f"pwt{dc}") for dc in range(DC)]
        post_sb = [cpool.tile([P, S], BF16, tag=f"post{ec}", name=f"post{ec}") for ec in range(DC)]
        for dc in range(DC):
            nc.sync.dma_start(out=wqt_sb[dc][:], in_=wqt_d[dc * P:(dc + 1) * P, :])
            nc.sync.dma_start(out=pwt_sb[dc][:], in_=pwt_d[dc * P:(dc + 1) * P, :])
            nc.sync.dma_start(out=post_sb[dc][:], in_=post_d[dc * P:(dc + 1) * P, :])

        # ---- phases A-D, quarter-pipelined ----
        dwpool = tc.alloc_tile_pool(name="kvdw", bufs=8, side="right")
        etpool = tc.alloc_tile_pool(name="evoT", bufs=1, side="right")
        evoT = [etpool.tile([P, PAD], BF16, tag=f"evoT{dc}", name=f"evoT{dc}") for dc in range(DC)]
        ebpool = tc.alloc_tile_pool(name="evbf", bufs=4, side="right")

        qpool = tc.alloc_tile_pool(name="qt", bufs=1, side="left")
        qt = [qpool.tile([P, S], BF16, tag=f"qt{ec}", name=f"qt{ec}") for ec in range(DC)]
        kvtpool = tc.alloc_tile_pool(name="kvt", bufs=1, side="left")
        kvpool = tc.alloc_tile_pool(name="kv", bufs=1, side="left")
        kvt = [kvtpool.tile([P, S], BF16, tag=f"kvt{ob}", name=f"kvt{ob}") for ob in range(DC)]
        kv = [kvpool.tile([P, D], BF16, tag=f"kv{sb}", name=f"kv{sb}") for sb in range(SB)]

        psa = tc.alloc_tile_pool(name="tpsA", bufs=2, space="PSUM")
        psb = tc.alloc_tile_pool(name="tpsB", bufs=2, space="PSUM")
        psc = tc.alloc_tile_pool(name="tpsC", bufs=2, space="PSUM")
        psd = tc.alloc_tile_pool(name="tpsD", bufs=2, space="PSUM")

        for dc in range(DC):
            nc.vector.memset(evoT[dc][:, 0:2].bitcast(F32), 0.0)
            nc.vector.memset(evoT[dc][:, 2 + S:PAD].bitcast(F32), 0.0)

        next_sb = 0
        for q in range(4):
            last_needed = min(4 * q + 4, SB - 1)
            while next_sb <= last_needed:
                sb = next_sb
                ev = epool.tile([P, D], F32, tag="evo", name="evo")
                nc.sync.dma_start(out=ev[:], in_=evo_d[sb * P:(sb + 1) * P, :])
                evb = ebpool.tile([P, D], BF16, tag="evb", name="evb")
                nc.scalar.copy(evb[:], ev[:])
                for dc in range(DC):
                    ps = psa.tile([P, P], BF16, tag="tp", name="tp")
                    nc.tensor.transpose(
                        ps[:], evb[:, dc * P:(dc + 1) * P], ident_bf[:]
                    )
                    nc.vector.tensor_copy(
                        evoT[dc][:, 2 + sb * P:2 + (sb + 1) * P], ps[:]
                    )
                next_sb += 1

            # B) depthwise conv for quarter q: 6 diag-matmuls into PSUM
            kvdw_q = [None] * DC
            for dc in range(DC):
                ps = psb.tile([P, 512], F32, tag="cps", name="cps")
                for k in range(KS):
                    nc.tensor.matmul(
                        ps[:], diagw_sb[dc * KS + k][:],
                        evoT[dc][:, q * 512 + k:q * 512 + k + 512],
                        start=(k == 0), stop=(k == KS - 1),
                    )
                kq = dwpool.tile([P, 512], BF16, tag="kvdwq", name="kvdwq")
                nc.scalar.copy(kq[:], ps[:])
                kvdw_q[dc] = kq

            # C) QT tile it=q (+pos/bq via identity matmul)
            it = q
            for ec in range(DC):
                ps = psc.tile([P, 512], F32, tag="qps", name="qps")
                for dc in range(DC):
                    nc.tensor.matmul(
                        ps[:], wqt_sb[dc][:, ec * P:(ec + 1) * P],
                        evoT[dc][:, 2 + it * 512:2 + (it + 1) * 512],
                        start=(dc == 0), stop=False,
                    )
                nc.tensor.matmul(
                    ps[:], ident_bf[:],
                    post_sb[ec][:, it * 512:(it + 1) * 512],
                    start=False, stop=True,
                )
                nc.scalar.copy(qt[ec][:, it * 512:(it + 1) * 512], ps[:])

            # D) KVT for st=q (pwb via ACT bias), then kv via PE transpose
            st = q
            for ob in range(DC):
                ps = psd.tile([P, 512], F32, tag="dps", name="kvtps")
                for dc in range(DC):
                    nc.tensor.matmul(
                        ps[:], pwt_sb[dc][:, ob * P:(ob + 1) * P],
                        kvdw_q[dc][:],
                        start=(dc == 0), stop=(dc == DC - 1),
                    )
                nc.scalar.activation(
                    kvt[ob][:, st * 512:(st + 1) * 512], ps[:],
                    ACT_ID, bias=pwbT[:, ob:ob + 1],
                )
            for sb in range(4 * q, 4 * q + 4):
                for ob in range(DC):
                    ps = psa.tile([P, P], BF16, tag="tp", name="tp2")
                    nc.tensor.transpose(
                        ps[:], kvt[ob][:, sb * P:(sb + 1) * P], ident_bf[:]
                    )
                    nc.vector.tensor_copy(
                        kv[sb][:, ob * P:(ob + 1) * P], ps[:]
                    )

        ebpool.release()
        etpool.release()
        dwpool.release()
        psd.release()
        psc.release()
        psb.release()
        psa.release()

        # ---- phase E: attention ----
        pss = tc.alloc_tile_pool(name="spsE", bufs=2, space="PSUM")
        pso = tc.alloc_tile_pool(name="opsE", bufs=4, space="PSUM")
        psl = tc.alloc_tile_pool(name="lpsE", bufs=2, space="PSUM")
        ptpool = tc.alloc_tile_pool(name="ptE", bufs=3, side="right")
        epipool = tc.alloc_tile_pool(name="epi", bufs=4, side="right")
        t12pool = tc.alloc_tile_pool(name="t12", bufs=4, side="right")
        for ig in range(IT):
            out_ps = [pso.tile([P, 512], F32, tag="ops", name="ops") for _ in range(4)]
            l_ps = psl.tile([P, 8], F32, tag="lps", name="lps")
            for jb in range(SB):
                s_ps = pss.tile([P, 512], F32, tag="sps", name="sps")
                for ec in range(DC):
                    nc.tensor.matmul(
                        s_ps[:], kvt[ec][:, jb * P:(jb + 1) * P],
                        qt[ec][:, ig * 512:(ig + 1) * 512],
                        start=(ec == 0), stop=(ec == DC - 1),
                    )
                p_t = ptpool.tile([P, 512], BF16, tag="pt", name="pt")
                nc.scalar.activation(p_t[:], s_ps[:], ACT_EXP, scale=scale)
                for ib in range(4):
                    nc.tensor.matmul(
                        out_ps[ib][:], p_t[:, ib * P:(ib + 1) * P],
                        kv[jb][:], start=(jb == 0), stop=False,
                    )
                    nc.tensor.matmul(
                        l_ps[:, 2 * ib:2 * ib + 2],
                        p_t[:, ib * P:(ib + 1) * P],
                        ones_col[:],
                        start=(jb == 0 and ib == 0),
                        stop=(jb == SB - 1 and ib == 3),
                    )
            for ib in range(4):
                sb = ig * 4 + ib
                l_sb = epipool.tile([P, 1], F32, tag="lsb", name="lsb")
                nc.vector.tensor_copy(l_sb[:], l_ps[:, 2 * ib:2 * ib + 1])
                rl = epipool.tile([P, 1], F32, tag="rl", name="rl")
                nc.vector.reciprocal(rl[:], l_ps[:, 2 * ib:2 * ib + 1])
                er = epipool.tile([P, 512], F32, tag="er", name="er")
                nc.sync.dma_start(out=er[:], in_=evo_d[sb * P:(sb + 1) * P, :])
                t1 = t12pool.tile([P, 512], BF16, tag="t1", name="t1")
                nc.scalar.activation(
                    t1[:], kv[sb][:], ACT_COPY, scale=l_sb[:]
                )
                t2 = t12pool.tile([P, 512], BF16, tag="t2", name="t2")
                nc.scalar.activation(
                    t2[:], er[:], ACT_COPY, scale=l_sb[:]
                )
                nc.tensor.matmul(
                    out_ps[ib][:], ident_bf[:], t1[:],
                    start=False, stop=False,
                )
                nc.tensor.matmul(
                    out_ps[ib][:], ident_bf[:], t2[:],
                    start=False, stop=True,
                )
                o_sb = epipool.tile([P, 512], F32, tag="osb", name="osb")
                nc.scalar.activation(o_sb[:], out_ps[ib][:], ACT_COPY, scale=rl[:])
                nc.sync.dma_start(out=out_d[sb * P:(sb + 1) * P, :], in_=o_sb[:])
        t12pool.release()
        epipool.release()
        ptpool.release()
        psl.release()
        pso.release()
        pss.release()
        kvpool.release()
        kvtpool.release()
        qpool.release()
        epool.release()
        cpool.release()

    _split_multi_waits(nc)
    return nc


def kernel(evo_local, Wq, bq, dw_w, dw_b, pw_w, pw_b, pos):
    evo_local = np.asarray(evo_local, dtype=np.float32)
    Wq = np.asarray(Wq, dtype=np.float32)
    bq = np.asarray(bq, dtype=np.float32)
    dw_w = np.asarray(dw_w, dtype=np.float32)
    dw_b = np.asarray(dw_b, dtype=np.float32)
    pw_w = np.asarray(pw_w, dtype=np.float32)
    pw_b = np.asarray(pw_b, dtype=np.float32)
    pos = np.asarray(pos, dtype=np.float32)

    if "nc" not in _COMPILED:
        _COMPILED["nc"] = _build()
    nc = _COMPILED["nc"]

    bf = ml_dtypes.bfloat16
    wqt = np.ascontiguousarray(Wq.T).astype(bf)               # [d, e]
    post = np.ascontiguousarray(pos[0].T + bq[:, None]).astype(bf)  # [e, s]
    pwt = np.ascontiguousarray(pw_w.T).astype(bf)             # [d, o]
    pwb_eff = (pw_b + pw_w @ dw_b).astype(np.float32)         # fold depthwise bias
    diagw = np.zeros((DC * KS * P, P), dtype=bf)
    w = dw_w[:, 0, :]                                          # [D, K]
    for dc in range(DC):
        for k in range(KS):
            t = dc * KS + k
            np.fill_diagonal(diagw[t * P:(t + 1) * P, :], w[dc * P:(dc + 1) * P, k].astype(bf))

    idbf = np.eye(P, dtype=bf)
    onesbf = np.ones((P, 2), dtype=bf)
    in_maps = []
    for c in range(N_CORES):
        in_maps.append({
            "evo": np.ascontiguousarray(evo_local[c]),
            "wqt": wqt,
            "post": post,
            "diagw": diagw,
            "pwt": pwt,
            "pwb": pwb_eff,
            "idbf": idbf,
            "ones": onesbf,
        })
    res = run_bass_kernel_spmd(nc, in_maps, core_ids=list(range(N_CORES)))
    out = np.stack([res.results[c]["out"] for c in range(N_CORES)], axis=0)
    return out.astype(np.float32)


# revision 10
# speedup vs baseline: 2.3472x; 1.1568x over previous
"""Trainium2 Bass kernel for nn_CrossAttention_45724221833727.

Data-parallel over batch: 8 samples -> 8 NeuronCores, one [S=2048, D=512]
cross-attention problem per core. Weights/pos replicated.

v2: everything dtype-bf16 on PE inputs; depthwise conv runs ON THE PE as
6 diagonal-matrix matmuls accumulating in PSUM (kills the GpSimd/DVE
elementwise conv that dominated v1); pos+bq folded host-side into a
transposed table added via an identity matmul; epilogue residuals folded
into the attention PSUM via l-scaled identity matmuls.

Per-core pipeline:
  A) load evo f32, ACT-cast to bf16, PE-transpose to evoT [d, s+pad]
  B) kvdw[d,s] = depthwise conv: sum_k diag(w_k) @ evoT(shift k) in PSUM
  C) QT[e, i] = Wq^T-matmuls(evoT) + ident-matmul(posT + bq)
  D) KVT[o, s] = pw^T-matmuls(kvdw) (+pwb via ACT bias);
     kv[s, o] = PE-transpose(KVT)
  E) attention: scoresT[j, i] -> exp (fused scale, no max-subtract;
     inputs are sub-unit-variance so |scores|*scale < ~3) -> PV + row-sum
     l in PSUM over j -> out_ps += l*kv + l*evo (ident matmuls)
     -> out = out_ps / l
"""

import math

import numpy as np
import ml_dtypes

import concourse.bass as bass
import concourse.mybir as mybir
import concourse.tile as tile
from concourse.bass_utils import run_bass_kernel_spmd
from concourse.masks import make_identity

F32 = mybir.dt.float32
F32R = mybir.dt.float32r
BF16 = mybir.dt.bfloat16
F8 = mybir.dt.float8e4
DR = mybir.MatmulPerfMode.DoubleRow
P = 128
S = 2048
D = 512
KS = 6
N_CORES = 8
SB = S // P      # 16 s-blocks
DC = D // P      # 4 d-chunks
IT = S // 512    # 4 i-tiles of 512
ACT_EXP = mybir.ActivationFunctionType.Exp
ACT_COPY = mybir.ActivationFunctionType.Copy
ACT_ID = mybir.ActivationFunctionType.Identity

_COMPILED = {}


def _install_tail_drain_patch():
    """This container's walrus build only accepts ONE sync wait per
    instruction; TileContext's tail drain carries one wait per live
    engine/DMA-queue. Split them across single-wait NOPs."""
    if getattr(tile.TileContext, "_tail_patch_installed", False):
        return

    def _patched_drain_and_barrier(self, tick_clock, wait_clock):
        from concourse.tile import ScopedClock

        drain_inst = self.nc.sync.drain()
        wait_clock.add_sem_waits(
            drain_inst.ins, ScopedClock({None: tick_clock.global_clock})
        )
        si = drain_inst.ins.sync_info
        waits = list(si.on_wait) if si and si.on_wait else []
        if len(waits) > 1:
            drain_inst.ins.sync_info = mybir.SyncInfo(
                on_wait=[], on_update=list(si.on_update or [])
            )
            for i, w in enumerate(waits):
                nop = self.nc.sync.nop(nofuse=True, hint=f"tail_wait_{i}")
                nop.ins.sync_info = mybir.SyncInfo(on_wait=[w], on_update=[])

        self.nc.all_engine_barrier()
        assert self.sems is not None
        popped = self.nc._tile_sem_poison_stack.pop()
        assert popped is self._sem_poison
        self.nc.clear_and_free_semaphores(list(self.sems.allocated().values()))
        self.nc.all_engine_barrier()

    tile.TileContext._drain_and_barrier = _patched_drain_and_barrier
    tile.TileContext._tail_patch_installed = True


def _split_multi_waits(nc):
    """Walrus in this container accepts at most ONE sync wait per
    instruction. Hoist extra waits onto single-wait NOPs inserted just
    before the instruction in the same engine's stream (equivalent
    semantics: the engine stalls at the NOP instead)."""
    ctr = [0]
    for fn in nc.m.functions:
        for blk in fn.blocks:
            insts = list(blk.instructions)
            out = []
            changed = False
            for inst in insts:
                si = inst.sync_info
                if si is not None and si.on_wait and len(si.on_wait) > 1:
                    waits = list(si.on_wait)
                    for w in waits[:-1]:
                        nop = mybir.InstNoOp(
                            name=f"splitw-{ctr[0]}", ins=[], outs=[]
                        )
                        ctr[0] += 1
                        nop.engine = inst.engine
                        nop.sync_info = mybir.SyncInfo(on_wait=[w], on_update=[])
                        out.append(nop)
                    inst.sync_info = mybir.SyncInfo(
                        on_wait=[waits[-1]], on_update=list(si.on_update or [])
                    )
                    changed = True
                out.append(inst)
            if changed:
                blk.instructions = out
    return nc


def _build():
    _install_tail_drain_patch()
    nc = bass.Bass()
    evo_d = nc.dram_tensor("evo", [S, D], F32, kind="ExternalInput")
    wqt_d = nc.dram_tensor("wqt", [D, D], BF16, kind="ExternalInput")   # Wq.T [d, e]
    post_d = nc.dram_tensor("post", [D, S], BF16, kind="ExternalInput")  # pos.T + bq [e, s]
    diagw_d = nc.dram_tensor("diagw", [DC * KS * P, P], BF16, kind="ExternalInput")
    pwt_d = nc.dram_tensor("pwt", [D, D], BF16, kind="ExternalInput")   # pw_w.T [d, o]
    pwb_d = nc.dram_tensor("pwb", [D], F32, kind="ExternalInput")       # pw_b + pw_w@dw_b
    idbf_d = nc.dram_tensor("idbf", [P, P], BF16, kind="ExternalInput")  # identity
    idf8_d = nc.dram_tensor("idf8", [P, P], F8, kind="ExternalInput")
    ones_d = nc.dram_tensor("ones", [P, 4], F8, kind="ExternalInput")
    out_d = nc.dram_tensor("out", [S, D], F32, kind="ExternalOutput")

    scale = 1.0 / math.sqrt(float(D))
    PAD = 2056  # 2 left pad + 2048 + 3 right pad, rounded up

    with tile.TileContext(nc) as tc:
        cpool = tc.alloc_tile_pool(name="consts", bufs=1, side="left")
        epool = tc.alloc_tile_pool(name="evo", bufs=6, side="left")

        # ---- constants ----
        ident_bf = cpool.tile([P, P], BF16, tag="identbf")
        nc.sync.dma_start(out=ident_bf[:], in_=idbf_d[:, :])
        ident_f8 = cpool.tile([P, P], F8, tag="identf8")
        nc.sync.dma_start(out=ident_f8[:], in_=idf8_d[:, :])
        ones_col = cpool.tile([P, 4], F8, tag="ones_col")
        nc.sync.dma_start(out=ones_col[:], in_=ones_d[:, :])
        pwbT = cpool.tile([P, DC], F32, tag="pwbT")
        nc.sync.dma_start(out=pwbT[:], in_=pwb_d.rearrange("(c p) -> p c", p=P))
        diagw_sb = [
            cpool.tile([P, P], BF16, tag=f"dg{t}", name=f"dg{t}")
            for t in range(DC * KS)
        ]
        for t in range(DC * KS):
            nc.sync.dma_start(
                out=diagw_sb[t][:], in_=diagw_d[t * P:(t + 1) * P, :]
            )
        wqt_sb = [cpool.tile([P, D], BF16, tag=f"wqt{dc}", name=f"wqt{dc}") for dc in range(DC)]
        pwt_sb = [cpool.tile([P, D], BF16, tag=f"pwt{dc}", name=f"pwt{dc}") for dc in range(DC)]
        post_sb = [cpool.tile([P, S], BF16, tag=f"post{ec}", name=f"post{ec}") for ec in range(DC)]
        for dc in range(DC):
            nc.sync.dma_start(out=wqt_sb[dc][:], in_=wqt_d[dc * P:(dc + 1) * P, :])
            nc.sync.dma_start(out=pwt_sb[dc][:], in_=pwt_d[dc * P:(dc + 1) * P, :])
            nc.sync.dma_start(out=post_sb[dc][:], in_=post_d[dc * P:(dc + 1) * P, :])

        # ---- phases A-D, quarter-pipelined ----
        dwpool = tc.alloc_tile_pool(name="kvdw", bufs=8, side="right")
        etpool = tc.alloc_tile_pool(name="evoT", bufs=1, side="right")
        evoT = [etpool.tile([P, PAD], BF16, tag=f"evoT{dc}", name=f"evoT{dc}") for dc in range(DC)]
        ebpool = tc.alloc_tile_pool(name="evbf", bufs=4, side="right")

        qpool = tc.alloc_tile_pool(name="qt", bufs=1, side="left")
        qt_all = qpool.tile([P, DC * S], F8, tag="qta", name="qta")
        kvtpool = tc.alloc_tile_pool(name="kvt", bufs=1, side="left")
        kvpool = tc.alloc_tile_pool(name="kv", bufs=1, side="left")
        kvt_all = kvtpool.tile([P, DC * S], F8, tag="kvta", name="kvta")
        kv_pair = [kvpool.tile([P, 2 * D], F8, tag=f"kv{jp}", name=f"kv{jp}")
                   for jp in range(SB // 2)]

        psa = tc.alloc_tile_pool(name="tpsA", bufs=2, space="PSUM")
        psg = tc.alloc_tile_pool(name="tpsG", bufs=3, space="PSUM")

        for dc in range(DC):
            nc.vector.memset(evoT[dc][:, 0:2].bitcast(F32), 0.0)
            nc.vector.memset(evoT[dc][:, 2 + S:PAD].bitcast(F32), 0.0)

        next_sb = 0
        for q in range(4):
            last_needed = min(4 * q + 4, SB - 1)
            while next_sb <= last_needed:
                sb = next_sb
                ev = epool.tile([P, D], F32, tag="evo", name="evo")
                nc.sync.dma_start(out=ev[:], in_=evo_d[sb * P:(sb + 1) * P, :])
                evb = ebpool.tile([P, D], BF16, tag="evb", name="evb")
                nc.scalar.copy(evb[:], ev[:])
                for dc in range(DC):
                    ps = psa.tile([P, P], BF16, tag="tp", name="tp")
                    nc.tensor.transpose(
                        ps[:], evb[:, dc * P:(dc + 1) * P], ident_bf[:]
                    )
                    nc.vector.tensor_copy(
                        evoT[dc][:, 2 + sb * P:2 + (sb + 1) * P], ps[:]
                    )
                next_sb += 1

            # B) depthwise conv for quarter q: 6 diag-matmuls into PSUM
            kvdw_q = [None] * DC
            for dc in range(DC):
                ps = psg.tile([P, 512], F32, tag="acc", name="cps")
                for k in range(KS):
                    nc.tensor.matmul(
                        ps[:], diagw_sb[dc * KS + k][:],
                        evoT[dc][:, q * 512 + k:q * 512 + k + 512],
                        start=(k == 0), stop=(k == KS - 1),
                    )
                kq = dwpool.tile([P, 512], BF16, tag="kvdwq", name="kvdwq")
                nc.scalar.copy(kq[:], ps[:])
                kvdw_q[dc] = kq

            # C) QT tile it=q (+pos/bq via identity matmul)
            it = q
            for ec in range(DC):
                ps = psg.tile([P, 512], F32, tag="acc", name="qps")
                for dc in range(DC):
                    nc.tensor.matmul(
                        ps[:], wqt_sb[dc][:, ec * P:(ec + 1) * P],
                        evoT[dc][:, 2 + it * 512:2 + (it + 1) * 512],
                        start=(dc == 0), stop=False,
                    )
                nc.tensor.matmul(
                    ps[:], ident_bf[:],
                    post_sb[ec][:, it * 512:(it + 1) * 512],
                    start=False, stop=True,
                )
                nc.scalar.copy(qt_all[:, ec * S + it * 512:ec * S + (it + 1) * 512], ps[:])

            # D) KVT for st=q (pwb via ACT bias), then kv via PE transpose
            st = q
            for ob in range(DC):
                ps = psg.tile([P, 512], F32, tag="acc", name="kvtps")
                for dc in range(DC):
                    nc.tensor.matmul(
                        ps[:], pwt_sb[dc][:, ob * P:(ob + 1) * P],
                        kvdw_q[dc][:],
                        start=(dc == 0), stop=(dc == DC - 1),
                    )
                nc.scalar.activation(
                    kvt_all[:, ob * S + st * 512:ob * S + (st + 1) * 512], ps[:],
                    ACT_ID, bias=pwbT[:, ob:ob + 1],
                )
            for sb in range(4 * q, 4 * q + 4):
                for ob in range(DC):
                    ps = psa.tile([P, 2 * P], F8, tag="tp8", name="tp2")
                    nc.tensor.transpose(
                        ps[:, 0:2 * P:2],
                        kvt_all[:, ob * S + sb * P:ob * S + (sb + 1) * P],
                        ident_f8[:]
                    )
                    nc.vector.tensor_copy(
                        kv_pair[sb // 2][:, (sb % 2) * D + ob * P:(sb % 2) * D + (ob + 1) * P],
                        ps[:, 0:2 * P:2]
                    )

        ebpool.release()
        etpool.release()
        dwpool.release()
        psg.release()
        psa.release()

        # ---- phase E: attention ----
        pss = tc.alloc_tile_pool(name="spsE", bufs=2, space="PSUM")
        pso = tc.alloc_tile_pool(name="opsE", bufs=4, space="PSUM")
        psl = tc.alloc_tile_pool(name="lpsE", bufs=2, space="PSUM")
        ptpool = tc.alloc_tile_pool(name="ptE", bufs=3, side="right")
        epipool = tc.alloc_tile_pool(name="epi", bufs=4, side="right")
        t12pool = tc.alloc_tile_pool(name="t12", bufs=4, side="right")
        kvt3 = kvt_all[:].rearrange("p (c n) -> p c n", n=S)
        qt3 = qt_all[:].rearrange("p (c n) -> p c n", n=S)
        ones3 = ones_col[:].rearrange("p (c n) -> p c n", n=2)
        for ig in range(IT):
            out_ps = [pso.tile([P, 512], F32, tag="ops", name="ops") for _ in range(4)]
            l_ps = psl.tile([P, 8], F32, tag="lps", name="lps")
            for jp in range(SB // 2):
                p_pair = ptpool.tile([P, 1024], F8, tag="pt", name="pt")
                for half in range(2):
                    jb = 2 * jp + half
                    s_ps = pss.tile([P, 512], F32, tag="sps", name="sps")
                    for ep in range(2):
                        nc.tensor.matmul(
                            s_ps[:],
                            kvt3[:, 2 * ep:2 * ep + 2, jb * P:(jb + 1) * P],
                            qt3[:, 2 * ep:2 * ep + 2, ig * 512:(ig + 1) * 512],
                            start=(ep == 0), stop=(ep == 1),
                            perf_mode=DR,
                        )
                    nc.scalar.activation(
                        p_pair[:, half * 512:(half + 1) * 512], s_ps[:],
                        ACT_EXP, scale=scale,
                    )
                p3 = p_pair[:].rearrange("p (c n) -> p c n", n=512)
                kv3 = kv_pair[jp][:].rearrange("p (c n) -> p c n", n=D)
                for ib in range(4):
                    nc.tensor.matmul(
                        out_ps[ib][:], p3[:, :, ib * P:(ib + 1) * P], kv3,
                        start=(jp == 0), stop=False,
                        perf_mode=DR,
                    )
                    nc.tensor.matmul(
                        l_ps[:, 2 * ib:2 * ib + 2],
                        p3[:, :, ib * P:(ib + 1) * P],
                        ones3,
                        start=(jp == 0 and ib == 0),
                        stop=(jp == SB // 2 - 1 and ib == 3),
                        perf_mode=DR,
                    )
            for ib in range(4):
                sb = ig * 4 + ib
                l_sb = epipool.tile([P, 1], F32, tag="lsb", name="lsb")
                nc.vector.tensor_copy(l_sb[:], l_ps[:, 2 * ib:2 * ib + 1])
                rl = epipool.tile([P, 1], F32, tag="rl", name="rl")
                nc.vector.reciprocal(rl[:], l_ps[:, 2 * ib:2 * ib + 1])
                er = epipool.tile([P, 512], F32, tag="er", name="er")
                nc.sync.dma_start(out=er[:], in_=evo_d[sb * P:(sb + 1) * P, :])
                t1 = t12pool.tile([P, 512], BF16, tag="t1", name="t1")
                nc.scalar.activation(
                    t1[:], kv_pair[sb // 2][:, (sb % 2) * D:(sb % 2 + 1) * D],
                    ACT_COPY, scale=l_sb[:]
                )
                t2 = t12pool.tile([P, 512], BF16, tag="t2", name="t2")
                nc.scalar.activation(
                    t2[:], er[:], ACT_COPY, scale=l_sb[:]
                )
                nc.tensor.matmul(
                    out_ps[ib][:], ident_bf[:], t1[:],
                    start=False, stop=False,
                )
                nc.tensor.matmul(
                    out_ps[ib][:], ident_bf[:], t2[:],
                    start=False, stop=True,
                )
                o_sb = epipool.tile([P, 512], F32, tag="osb", name="osb")
                nc.scalar.activation(o_sb[:], out_ps[ib][:], ACT_COPY, scale=rl[:])
                nc.sync.dma_start(out=out_d[sb * P:(sb + 1) * P, :], in_=o_sb[:])
        t12pool.release()
        epipool.release()
        ptpool.release()
        psl.release()
        pso.release()
        pss.release()
        kvpool.release()
        kvtpool.release()
        qpool.release()
        epool.release()
        cpool.release()

    _split_multi_waits(nc)
    return nc


def kernel(evo_local, Wq, bq, dw_w, dw_b, pw_w, pw_b, pos):
    evo_local = np.asarray(evo_local, dtype=np.float32)
    Wq = np.asarray(Wq, dtype=np.float32)
    bq = np.asarray(bq, dtype=np.float32)
    dw_w = np.asarray(dw_w, dtype=np.float32)
    dw_b = np.asarray(dw_b, dtype=np.float32)
    pw_w = np.asarray(pw_w, dtype=np.float32)
    pw_b = np.asarray(pw_b, dtype=np.float32)
    pos = np.asarray(pos, dtype=np.float32)

    if "nc" not in _COMPILED:
        _COMPILED["nc"] = _build()
    nc = _COMPILED["nc"]

    bf = ml_dtypes.bfloat16
    wqt = np.ascontiguousarray(Wq.T).astype(bf)               # [d, e]
    post = np.ascontiguousarray(pos[0].T + bq[:, None]).astype(bf)  # [e, s]
    pwt = np.ascontiguousarray(pw_w.T).astype(bf)             # [d, o]
    pwb_eff = (pw_b + pw_w @ dw_b).astype(np.float32)         # fold depthwise bias
    diagw = np.zeros((DC * KS * P, P), dtype=bf)
    w = dw_w[:, 0, :]                                          # [D, K]
    for dc in range(DC):
        for k in range(KS):
            t = dc * KS + k
            np.fill_diagonal(diagw[t * P:(t + 1) * P, :], w[dc * P:(dc + 1) * P, k].astype(bf))

    idbf = np.eye(P, dtype=bf)
    idf8 = np.eye(P, dtype=ml_dtypes.float8_e4m3)
    onesbf = np.ones((P, 4), dtype=ml_dtypes.float8_e4m3)
    in_maps = []
    for c in range(N_CORES):
        in_maps.append({
            "evo": np.ascontiguousarray(evo_local[c]),
            "wqt": wqt,
            "post": post,
            "diagw": diagw,
            "pwt": pwt,
            "pwb": pwb_eff,
            "idbf": idbf,
            "idf8": idf8,
            "ones": onesbf,
        })
    res = run_bass_kernel_spmd(nc, in_maps, core_ids=list(range(N_CORES)))
    out = np.stack([res.results[c]["out"] for c in range(N_CORES)], axis=0)
    return out.astype(np.float32)
